# revision 3
# baseline (speedup 1.0000x reference)
"""ChebGCN (K=5, 5 layers) on 8 Trainium2 NeuronCores via Bass.

Strategy (graph/data parallel, per sharding hint):
- Nodes sharded by range across 8 cores (12500/core, padded to 12544 = 128*98).
- Per core, nodes are degree-sorted and blocked into 98 windows of 128 nodes;
  window w / lane p maps to DRAM row p*98+w of the rank's y-buffer (so SBUF
  [128, 98, 128] tiles load/store with contiguous per-partition rows).
- All tensors carried in "y-space": y_k = norm * x_k, which makes the Cheb
  recurrence y_k = -(2)norm^2 * A(y_{k-1}) - y_{k-2} with A = plain
  gather/segment-sum over edges, and (since leaky_relu is positively
  homogeneous and norm > 0) the dense layer becomes
  y0_next = lrelu(sum_k y_k @ W_k + norm x bc) with no divisions.
- Per Chebyshev step: AllGather y (bf16, 25.7 MB) -> per-core dma_gather of
  edge-source rows into rectangular per-(window, bucket) slot grids
  (int16 indices => 4 buckets of 25088 u-rows) -> DVE fold-tree segment-sum
  -> scale/subtract -> store shard.
- Dense layers: PE matmuls, bases transposed on load via DMA-transpose;
  bias folded in as a K=1 matmul (exact for bc=0 and correct for bc!=0 in
  y-space).

kernel(**inputs) takes the full-size numpy inputs and returns [100000, 8] f32.
"""

import sys

for _p in ("/opt/trn_rl_repo", "/root/problem"):
    if _p not in sys.path:
        sys.path.insert(0, _p)

import os
import numpy as np
import ml_dtypes

# ---------------------------------------------------------------- config ----

N_CORES = 8
F = 128
H = 128
NCLS = 8
K = 5
NLAYERS = 5
NBUCK = 4
CHUNK_BLOCKS = 64  # max 128-row column-blocks per dma_gather (8192 idxs)


class Cfg:
    def __init__(self, n_nodes, n_edges, windows):
        self.n_nodes = n_nodes
        self.n_edges = n_edges
        self.W = windows                      # windows per core
        self.shard = n_nodes // N_CORES       # real nodes per core
        self.padn = 128 * windows             # padded nodes per core
        assert self.shard <= self.padn
        self.brows = 2 * self.padn            # u-rows per bucket (2 ranks)
        assert self.brows <= 32768, "int16 gather index range"
        self.urows = N_CORES * self.padn
        self.zero_idx = self.padn - 1         # per-bucket-relative zero row
        # (row padn-1 of the bucket's first rank == sorted-last pad node)


FULL = Cfg(100000, 3200000, 98)


# ---------------------------------------------------------- preprocessing ----

def preprocess(cfg, x, src, dst):
    """Host-side graph preprocessing. Returns (geometry, per-core arrays)."""
    n, sh, W, padn = cfg.n_nodes, cfg.shard, cfg.W, cfg.padn
    deg = np.bincount(dst, minlength=n).astype(np.float32)
    norm = np.clip(deg, 1.0, None) ** -0.5

    # per-core degree sort -> (p, w) assignment and DRAM row
    pos_of_node = np.empty(n, dtype=np.int64)   # row within rank block
    wp_of_node = np.empty((n, 2), dtype=np.int32)  # (w, p)
    orders = []
    for c in range(N_CORES):
        base = c * sh
        deg_c = deg[base : base + sh]
        order = np.argsort(-deg_c, kind="stable")  # sorted node list
        orders.append(order)
        i = np.arange(sh)
        # sorted index i -> (w=i//128, p=i%128), row = p*W + w
        w_i = i // 128
        p_i = i % 128
        pos_of_node[base + order] = p_i * W + w_i
        wp_of_node[base + order, 0] = w_i
        wp_of_node[base + order, 1] = p_i
    urow_of_node = (np.arange(n) // sh) * padn + pos_of_node

    src_urow = urow_of_node[src]
    src_bucket = (src_urow // cfg.brows).astype(np.int32)
    src_rel = (src_urow - src_bucket * cfg.brows).astype(np.int32)

    dst_core = dst // sh
    dst_w = wp_of_node[dst, 0]
    dst_p = wp_of_node[dst, 1]

    # per (core, bucket, w, p) counts -> shared geometry g[b][w]
    key = ((dst_core.astype(np.int64) * NBUCK + src_bucket) * W + dst_w) * 128 + dst_p
    cnt = np.bincount(key, minlength=N_CORES * NBUCK * W * 128).reshape(
        N_CORES, NBUCK, W, 128
    )
    g0 = cnt.max(axis=(0, 3))  # [NBUCK, W]
    g = ((g0 + 1) // 2 * 2).clip(2, None)  # even, >=2
    # monotone non-increasing in w (degree-sorted windows)
    g = np.maximum.accumulate(g[:, ::-1], axis=1)[:, ::-1]
    assert g.max() <= CHUNK_BLOCKS, f"per-bucket node degree too high: {g.max()}"

    # chunks: runs of equal g split to <= CHUNK_BLOCKS column-blocks
    chunks = []  # (b, w0, nw, g, col_off, slot_base)
    col_off = 0  # int16 columns (16 idx each) into the concatenated idx array
    slot_base = 0
    bucket_slotbase = []
    for b in range(NBUCK):
        bucket_slotbase.append(slot_base)
        w0 = 0
        while w0 < W:
            gv = int(g[b, w0])
            w1 = w0
            while w1 < W and int(g[b, w1]) == gv and (w1 - w0 + 1) * gv <= CHUNK_BLOCKS:
                w1 += 1
            nw = w1 - w0
            slots = nw * gv * 128
            chunks.append((b, w0, nw, gv, col_off, slot_base))
            col_off += slots // 16
            slot_base += slots
            w0 = w1
    total_slots = slot_base
    total_cols = col_off

    # per-core slot index arrays
    cum_g = np.zeros((NBUCK, W + 1), dtype=np.int64)
    for b in range(NBUCK):
        cum_g[b, 1:] = np.cumsum(g[b])
    per_core = []
    for c in range(N_CORES):
        m = dst_core == c
        eb = src_bucket[m]
        ew = dst_w[m].astype(np.int64)
        ep = dst_p[m].astype(np.int64)
        erel = src_rel[m]
        # j = rank of edge within its (b, w, p) group
        gkey = (eb * W + ew) * 128 + ep
        sort = np.argsort(gkey, kind="stable")
        gs = gkey[sort]
        grp_start = np.zeros(len(gs), dtype=np.int64)
        newg = np.ones(len(gs), dtype=bool)
        newg[1:] = gs[1:] != gs[:-1]
        starts = np.flatnonzero(newg)
        grp_start[starts] = np.arange(len(gs))[starts]
        grp_start = np.maximum.accumulate(grp_start)
        j = np.arange(len(gs)) - grp_start
        # slot = bucket_base + (cum_g[b, w] + j) * 128 + p
        sb = np.asarray(bucket_slotbase)[eb[sort]]
        slot = sb + (cum_g[eb[sort], ew[sort]] + j) * 128 + ep[sort]
        idxarr = np.full(total_slots, cfg.zero_idx, dtype=np.int16)
        idxarr[slot] = erel[sort].astype(np.int16)
        idx16 = idxarr.reshape(-1, 16).T.copy()  # [16, total_cols]
        per_core.append(idx16)

    geom = dict(
        g=g,
        chunks=chunks,
        total_cols=total_cols,
        total_slots=total_slots,
        bucket_slotbase=bucket_slotbase,
    )

    # per-core dense/per-node constants
    consts = []
    for c in range(N_CORES):
        base = c * sh
        order = orders[c]
        node_at_i = np.full(cfg.padn, -1, dtype=np.int64)
        node_at_i[: sh] = base + order
        # (p, w) grid values
        nrm = np.zeros((128, W), dtype=np.float32)
        i = np.arange(cfg.padn)
        wi, pi = i // 128, i % 128
        valid = node_at_i >= 0
        nrm[pi[valid], wi[valid]] = norm[node_at_i[valid]]
        s1 = -(nrm**2)
        s2 = 2.0 * s1
        invn = np.zeros((128, W), dtype=np.float32)
        invn[pi[valid], wi[valid]] = 1.0 / norm[node_at_i[valid]]
        # norm by column index w*128+p (lhsT order for bias matmul)
        normb = np.zeros((1, cfg.padn), dtype=np.float32)
        normb[0, wi[valid] * 128 + pi[valid]] = norm[node_at_i[valid]]
        # x rows permuted to DRAM layout (row p*W+w)
        xp = np.zeros((cfg.padn, F), dtype=np.float32)
        xp[pos_of_node[base : base + sh]] = x[base : base + sh]
        consts.append(
            dict(
                x_perm=xp,
                s1=s1,
                s2=s2,
                invn=invn,
                normv=nrm,
                normb=normb.astype(ml_dtypes.bfloat16),
                idx16=per_core[c],
                pos=pos_of_node[base : base + sh].copy(),
            )
        )
    return geom, consts


# ------------------------------------------------------------ bass program ----

def build(cfg, geom):
    import concourse.bacc as bacc
    import concourse.mybir as mybir
    import concourse.tile as tile

    W = cfg.W
    padn = cfg.padn
    bf16 = mybir.dt.bfloat16
    f32 = mybir.dt.float32
    TOTCOLS = geom["total_cols"]

    nc = bacc.Bacc("TRN2")
    x_in = nc.dram_tensor("x_perm", [padn, F], f32, kind="ExternalInput")
    idx_in = nc.dram_tensor("idx16", [16, TOTCOLS], mybir.dt.int16, kind="ExternalInput")
    s1_in = nc.dram_tensor("s1", [128, W], f32, kind="ExternalInput")
    s2_in = nc.dram_tensor("s2", [128, W], f32, kind="ExternalInput")
    invn_in = nc.dram_tensor("invn", [128, W], f32, kind="ExternalInput")
    normv_in = nc.dram_tensor("normv", [128, W], f32, kind="ExternalInput")
    normb_in = nc.dram_tensor("normb", [1, padn], bf16, kind="ExternalInput")
    wc_in = nc.dram_tensor("wc", [128, NLAYERS * K * H], bf16, kind="ExternalInput")
    bct_in = nc.dram_tensor("bct", [1, NLAYERS * H], bf16, kind="ExternalInput")
    wout_in = nc.dram_tensor("wout", [128, NCLS], bf16, kind="ExternalInput")
    bout_in = nc.dram_tensor("bout", [1, NCLS], bf16, kind="ExternalInput")
    logits_out = nc.dram_tensor("logits", [padn, NCLS], f32, kind="ExternalOutput")

    with tile.TileContext(nc) as tc:
        with (
            tc.tile_pool(name="dram", bufs=1, space="DRAM") as dram,
            tc.tile_pool(name="const", bufs=1) as constp,
            tc.tile_pool(name="aggp", bufs=1) as aggp,
            tc.tile_pool(name="state", bufs=1) as statep,
            tc.tile_pool(name="msgp", bufs=2) as msgp,
            tc.tile_pool(name="scrp", bufs=2) as scrp,
            tc.tile_pool(name="idxp", bufs=2) as idxp,
            tc.tile_pool(name="densep", bufs=3) as densep,
            tc.tile_pool(name="psum", bufs=4, space="PSUM") as psump,
        ):
            u = dram.tile([cfg.urows, F], bf16)
            ybuf = [dram.tile([padn, F], bf16, name=f"y{i}") for i in range(K)]
            idx_rep = dram.tile([128, TOTCOLS], mybir.dt.int16)

            # replicate idx to 128 partitions (device-side, once)
            for r in range(8):
                nc.sync.dma_start(idx_rep[16 * r : 16 * (r + 1), :], idx_in[:, :])

            # resident constants
            s1 = constp.tile([128, W], f32)
            s2 = constp.tile([128, W], f32)
            invn = constp.tile([128, W], f32)
            normv = constp.tile([128, W], f32)
            normb = constp.tile([1, padn], bf16)
            wc = constp.tile([128, NLAYERS * K * H], bf16)
            bct = constp.tile([1, NLAYERS * H], bf16)
            wout = constp.tile([128, NCLS], bf16)
            bout = constp.tile([1, NCLS], bf16)
            for t, src_ in (
                (s1, s1_in), (s2, s2_in), (invn, invn_in), (normv, normv_in),
                (normb, normb_in), (wc, wc_in), (bct, bct_in), (wout, wout_in),
                (bout, bout_in),
            ):
                nc.sync.dma_start(t[:], src_[:])

            agg = aggp.tile([128, W * F], f32)          # [p, w, f]
            ynew = statep.tile([128, W * F], bf16)      # [p, w, f]
            yprev = statep.tile([128, W * F], bf16)

            def shard_rows_ap(dram_t):
                # DRAM [padn, F] rows p*W+w  <->  [p, w, f]
                return dram_t[:].rearrange("(p w) f -> p (w f)", p=128)

            # ---- y0 = norm * x ----
            nc.sync.dma_start(agg[:], shard_rows_ap(x_in))
            nc.vector.tensor_tensor(
                out=ynew[:].rearrange("p (w f) -> p w f", f=F),
                in0=agg[:].rearrange("p (w f) -> p w f", f=F),
                in1=normv[:].to_broadcast([128, W, F]),
                op=mybir.AluOpType.mult,
            )
            nc.sync.dma_start(shard_rows_ap(ybuf[0]), ynew[:])

            core_ids = list(range(N_CORES))

            def spmm_step(k):
                """ybuf[k] = s_k * A(ybuf[k-1]) - (ybuf[k-2] if k>=2)."""
                nc.gpsimd.collective_compute(
                    "AllGather",
                    mybir.AluOpType.bypass,
                    ins=[ybuf[k - 1].opt()],
                    outs=[u.opt()],
                    replica_groups=[core_ids],
                )
                if k >= 2:
                    nc.sync.dma_start(yprev[:], shard_rows_ap(ybuf[k - 2]))
                for (b, w0, nw, gv, col_off, _sb) in geom["chunks"]:
                    blocks = nw * gv
                    slots = blocks * 128
                    idx_sb = idxp.tile([128, CHUNK_BLOCKS * 8], mybir.dt.int16, tag="idx")
                    nc.sync.dma_start(
                        idx_sb[:, : slots // 16],
                        idx_rep[:, col_off : col_off + slots // 16],
                    )
                    msg = msgp.tile([128, CHUNK_BLOCKS * F], bf16, tag="msg")
                    nc.gpsimd.dma_gather(
                        msg[:, : blocks * F].rearrange("p (c f) -> p c f", f=F),
                        u[b * cfg.brows : (b + 1) * cfg.brows, :],
                        idx_sb[:, : slots // 16],
                        slots,
                        slots,
                        F,
                        single_packet=False,
                    )
                    # fold-tree: [p, w, g, f] -> [p, w, 1, f]
                    msgv = msg[:, : blocks * F].rearrange(
                        "p (w g f) -> p w g f", w=nw, g=gv
                    )
                    h = gv // 2
                    scr = scrp.tile([128, (CHUNK_BLOCKS // 2) * F], f32, tag="scr")
                    scrv = scr[:, : nw * h * F].rearrange(
                        "p (w g f) -> p w g f", w=nw, g=h
                    )
                    nc.vector.tensor_tensor(
                        out=scrv,
                        in0=msgv[:, :, 0:h, :],
                        in1=msgv[:, :, h : 2 * h, :],
                        op=mybir.AluOpType.add,
                    )
                    cur = h
                    while cur > 1:
                        nh = cur - cur // 2
                        nc.vector.tensor_tensor(
                            out=scrv[:, :, 0 : cur // 2, :],
                            in0=scrv[:, :, 0 : cur // 2, :],
                            in1=scrv[:, :, nh : nh + cur // 2, :],
                            op=mybir.AluOpType.add,
                        )
                        cur = nh
                    part = scrv[:, :, 0, :]
                    aggv = agg[:, w0 * F : (w0 + nw) * F].rearrange(
                        "p (w f) -> p w f", f=F
                    )
                    if b == 0:
                        nc.vector.tensor_copy(out=aggv, in_=part)
                    else:
                        nc.vector.tensor_tensor(
                            out=aggv, in0=aggv, in1=part, op=mybir.AluOpType.add
                        )
                # evac: ynew = s*agg (- yprev)
                sv = (s1 if k == 1 else s2)[:].to_broadcast([128, W, F])
                nc.vector.tensor_tensor(
                    out=agg[:].rearrange("p (w f) -> p w f", f=F),
                    in0=agg[:].rearrange("p (w f) -> p w f", f=F),
                    in1=sv,
                    op=mybir.AluOpType.mult,
                )
                if k >= 2:
                    nc.vector.tensor_tensor(
                        out=ynew[:], in0=agg[:], in1=yprev[:],
                        op=mybir.AluOpType.subtract,
                    )
                else:
                    nc.vector.tensor_copy(out=ynew[:], in_=agg[:])
                nc.sync.dma_start(shard_rows_ap(ybuf[k]), ynew[:])

            def dense_layer(layer):
                """ynew = lrelu(sum_k y_k @ Wc[l,k] + norm (x) bc[l]); -> ybuf[0]."""
                for w in range(W):
                    ps = psump.tile([128, H], f32, tag="psd")
                    for kb in range(K):
                        ytr = densep.tile([128, 128], bf16, tag="ytr")
                        nc.sync.dma_start(
                            ytr[:],
                            ybuf[kb][:].rearrange("(p w) f -> p w f", p=128)[:, w, :],
                            transpose=True,
                        )
                        nc.tensor.matmul(
                            ps[:],
                            lhsT=ytr[:],
                            rhs=wc[:, (layer * K + kb) * H : (layer * K + kb + 1) * H],
                            start=(kb == 0),
                            stop=False,
                        )
                    nc.tensor.matmul(
                        ps[:],
                        lhsT=normb[0:1, w * 128 : (w + 1) * 128],
                        rhs=bct[0:1, layer * H : (layer + 1) * H],
                        start=False,
                        stop=True,
                    )
                    tmp = densep.tile([128, H], f32, tag="lrtmp")
                    nc.vector.tensor_scalar_mul(tmp[:], ps[:], 0.01)
                    nc.vector.tensor_tensor(
                        out=ynew[:, w * F : (w + 1) * F],
                        in0=ps[:],
                        in1=tmp[:],
                        op=mybir.AluOpType.max,
                    )
                nc.sync.dma_start(shard_rows_ap(ybuf[0]), ynew[:])

            outsb = statep.tile([128, W * NCLS], f32)

            def final_layer():
                for w in range(W):
                    ps = psump.tile([128, NCLS], f32, tag="psf")
                    ytr = densep.tile([128, 128], bf16, tag="ytr")
                    nc.sync.dma_start(
                        ytr[:],
                        ybuf[0][:].rearrange("(p w) f -> p w f", p=128)[:, w, :],
                        transpose=True,
                    )
                    nc.tensor.matmul(
                        ps[:], lhsT=ytr[:], rhs=wout[:], start=True, stop=False
                    )
                    nc.tensor.matmul(
                        ps[:],
                        lhsT=normb[0:1, w * 128 : (w + 1) * 128],
                        rhs=bout[0:1, :],
                        start=False,
                        stop=True,
                    )
                    nc.vector.tensor_scalar(
                        out=outsb[:, w * NCLS : (w + 1) * NCLS],
                        in0=ps[:],
                        scalar1=invn[:, w : w + 1],
                        scalar2=None,
                        op0=mybir.AluOpType.mult,
                    )
                nc.sync.dma_start(
                    logits_out[:].rearrange("(p w) c -> p (w c)", p=128), outsb[:]
                )

            for layer in range(NLAYERS):
                for k in range(1, K):
                    spmm_step(k)
                dense_layer(layer)
            final_layer()

    nc.compile()
    return nc


# ------------------------------------------------------------------ driver ----

_CACHE = {}


def _get_program(cfg, geom):
    key = "prog"
    if key not in _CACHE:
        import bir_fix

        bir_fix.install()
        _CACHE[key] = build(cfg, geom)
    return _CACHE[key]


def run(cfg, inputs, *, n_timing_runs=0):
    x = np.asarray(inputs["x"], np.float32)
    src = np.asarray(inputs["src"]).astype(np.int64)
    dst = np.asarray(inputs["dst"]).astype(np.int64)
    Wc = np.asarray(inputs["Wc"], np.float32)
    bc = np.asarray(inputs["bc"], np.float32)
    Wout = np.asarray(inputs["Wout"], np.float32)
    bout = np.asarray(inputs["bout"], np.float32)

    geom, consts = preprocess(cfg, x, src, dst)
    nc = _get_program(cfg, geom)

    # weights, shared across cores
    wc_host = np.zeros((128, NLAYERS * K * H), dtype=ml_dtypes.bfloat16)
    for l in range(NLAYERS):
        for kb in range(K):
            wc_host[:, (l * K + kb) * H : (l * K + kb + 1) * H] = Wc[
                l, kb * F : (kb + 1) * F, :
            ].astype(ml_dtypes.bfloat16)
    bct_host = np.zeros((1, NLAYERS * H), dtype=ml_dtypes.bfloat16)
    for l in range(NLAYERS):
        bct_host[0, l * H : (l + 1) * H] = bc[l].astype(ml_dtypes.bfloat16)
    wout_host = Wout.astype(ml_dtypes.bfloat16)
    bout_host = bout.reshape(1, NCLS).astype(ml_dtypes.bfloat16)

    in_maps = []
    for c in range(N_CORES):
        cc = consts[c]
        in_maps.append(
            {
                "x_perm": cc["x_perm"],
                "idx16": cc["idx16"],
                "s1": cc["s1"],
                "s2": cc["s2"],
                "invn": cc["invn"],
                "normv": cc["normv"],
                "normb": cc["normb"],
                "wc": wc_host,
                "bct": bct_host,
                "wout": wout_host,
                "bout": bout_host,
            }
        )

    from concourse.bass_utils import run_bass_kernel_spmd

    res = run_bass_kernel_spmd(nc, in_maps, list(range(N_CORES)))

    times = []
    if n_timing_runs:
        import time

        for _ in range(n_timing_runs):
            t0 = time.time()
            res = run_bass_kernel_spmd(nc, in_maps, list(range(N_CORES)))
            times.append(time.time() - t0)

    out = np.empty((cfg.n_nodes, NCLS), dtype=np.float32)
    for c in range(N_CORES):
        shard_out = res.results[c]["logits"]  # [padn, NCLS], rows p*W+w
        out[c * cfg.shard : (c + 1) * cfg.shard] = shard_out[consts[c]["pos"]]
    return (out, times) if n_timing_runs else out


def kernel(**inputs) -> np.ndarray:
    return run(FULL, inputs)
